# revision 15
# baseline (speedup 1.0000x reference)
"""Trainium2 Bass kernel for AttnAdaINCos (cosine-attention AdaIN style transfer).

Sharding: 8 cores = 4 batches x 2 halves. Within a pair the content-pixel
axis is split (sequence parallel) AND the style-token axis is split: core h
computes Gt'/Hs/B/B2/u/hsum partials over its half of the style tokens (the
halves arrive via the per-core input map, so the program is identical on
both cores) plus content stats for its pixel half.  One pairwise bf16
AllGather (DRAM bounce, ~15-20us fixed cost) exchanges B/B2/u/hsum/h2sum/bn
partials in a single shot; cheap local adds merge the two slots.

Key identity: cos in [-1, 1] by Cauchy-Schwarz, so the reference's
S = relu(cos + 1) = 1 + cos is LINEAR in cos — the attention collapses
associatively. With unit-normalized key features F'[d,p] (content) and
Gt'[j,d] (style, row-normalized, transposed) and style values Hs[j,c]:

  mean_num[c,p] = sum_j (1+cos_jp) Hs[j,c] = hsum[c] + (B^T F')[c,p],
      B[d,c] = sum_j Gt'[j,d] Hs[j,c]     [C x C, reduced across the pair]
  sq_num uses B2 = Gt'^T Hs^2 and h2sum
  den_p = M + u^T F'_p + EPS,  u[d] = sum_j Gt'[j,d]

  mean = mean_num/den + bh    (Hs bias cancels in std, shifts mean)
  std  = sqrt(sq_num/den - (mean-bh)^2)
  out = std * (ct - cmean)/cstd + mean    (cmean/cstd over the full image,
                                           merged from per-half bn partials)

Dtypes: all matrix products run fp8e4 DoubleRow on [128, K/256, 2, n]
interleaved operands with fp32 PSUM; the exchange runs bf16; combine fp32.
The PE p-state ramp rewards uninterrupted matmul streams, so inputs load
with few big DMAs up front and evacuations alternate scalar/vector/gpsimd.
The gpsimd queue is FIFO and the collective blocks it, so everything gpsimd
that is needed while the collective is in flight is emitted after it.
"""

import sys

if "/opt/trn_rl_repo" not in sys.path:
    sys.path.insert(0, "/opt/trn_rl_repo")

from contextlib import ExitStack

import numpy as np

import ml_dtypes

import concourse.bass as bass
import concourse.tile as tile
from concourse import bacc, mybir
from concourse.bass_utils import run_bass_kernel_spmd

F32 = mybir.dt.float32
BF16 = mybir.dt.bfloat16
FP8 = mybir.dt.float8e4
DR = mybir.MatmulPerfMode.DoubleRow
AF = mybir.ActivationFunctionType
ALU = mybir.AluOpType
PS = bass.MemorySpace.PSUM
EPS = 1e-5
NC = 512  # free-dim chunk size (one PSUM bank of fp32)

GROUPS = [[0, 1], [2, 3], [4, 5], [6, 7]]


def build_nc(C=512, N=4096, NL=2048):
    """Build the single SPMD program (identical on all cores)."""
    KB = C // 128     # channel 128-blocks (contraction blocks)
    CB = C // 128     # output-channel 128-blocks
    N2 = N // 2       # style tokens processed locally
    QB = N2 // 128    # local style-token 128-blocks (j-blocks)
    J2 = QB // 2      # local style-token 256-pair tiles
    PC = NL // NC     # local-pixel 512-chunks
    PW = PC // 2      # local-pixel 1024-wide combine groups
    NT = 2 * NL       # full-image pixel count (for content stats)
    K2 = KB // 2
    CCR = C + C + 5   # exchange rows: B, B2, hsum, h2sum, u, bn(2)

    nc = bacc.Bacc("TRN2", target_bir_lowering=False, num_devices=8)

    ck = nc.dram_tensor("ck", [128, K2, 2, NL], FP8, kind="ExternalInput")
    sk = nc.dram_tensor("sk", [128, K2, 2, N2], FP8, kind="ExternalInput")
    st = nc.dram_tensor("st", [128, K2, 2, N2], FP8, kind="ExternalInput")
    ct = nc.dram_tensor("ct", [C, NL], BF16, kind="ExternalInput")
    wf = nc.dram_tensor("wf", [128, K2, 2, C], FP8, kind="ExternalInput")
    wg = nc.dram_tensor("wg", [128, K2, 2, C], FP8, kind="ExternalInput")
    wh = nc.dram_tensor("wh", [128, K2, 2, C], FP8, kind="ExternalInput")
    bfb = nc.dram_tensor("bfb", [128, KB], F32, kind="ExternalInput")
    bgr = nc.dram_tensor("bgr", [1, C], BF16, kind="ExternalInput")
    out = nc.dram_tensor("out", [C, NL], BF16, kind="ExternalOutput")

    with tile.TileContext(nc) as tc:
        with ExitStack() as stk:
            # big 64B-multiple matmul operands first (keeps them aligned)
            rsk = stk.enter_context(tc.tile_pool(name="rsk", bufs=1))
            rct = stk.enter_context(tc.tile_pool(name="rct", bufs=1))
            resw = stk.enter_context(tc.tile_pool(name="resw", bufs=1))
            gtpool = stk.enter_context(tc.tile_pool(name="gtpool", bufs=1))
            hpool = stk.enter_context(tc.tile_pool(name="hpool", bufs=1))
            fspool = stk.enter_context(tc.tile_pool(name="fspool", bufs=1))
            bpool = stk.enter_context(tc.tile_pool(name="bpool", bufs=1))
            gbp = stk.enter_context(tc.tile_pool(name="gbp", bufs=2))
            stg = stk.enter_context(tc.tile_pool(name="stg", bufs=4))
            etmp = stk.enter_context(tc.tile_pool(name="etmp", bufs=4))
            ivp = stk.enter_context(tc.tile_pool(name="ivp", bufs=1))
            cmb = stk.enter_context(tc.tile_pool(name="cmb", bufs=2))
            op = stk.enter_context(tc.tile_pool(name="op", bufs=2))
            pp = stk.enter_context(tc.tile_pool(name="pp", bufs=8, space=PS))
            # odd-sized tiles last
            const = stk.enter_context(tc.tile_pool(name="const", bufs=1))
            small = stk.enter_context(tc.tile_pool(name="small", bufs=4))
            drp = stk.enter_context(
                tc.tile_pool(name="drp", bufs=1, space=bass.MemorySpace.DRAM))

            # ---- resident input tiles ----
            SKT = [rsk.tile([128, 2, N2], FP8, tag=f"SK{k2}", name=f"SK{k2}")
                   for k2 in range(K2)]
            STT = [rsk.tile([128, 2, N2], FP8, tag=f"ST{k2}", name=f"ST{k2}")
                   for k2 in range(K2)]
            CKT = [rsk.tile([128, 2, NL], FP8, tag=f"CK{k2}", name=f"CK{k2}")
                   for k2 in range(K2)]
            CT = [rct.tile([128, NL], BF16, tag=f"CT{cb}", name=f"CT{cb}")
                  for cb in range(CB)]
            wfS = resw.tile([128, K2, 2, C], FP8, tag="wf", name="wf_s")
            wgS = resw.tile([128, K2, 2, C], FP8, tag="wg", name="wg_s")
            whS = resw.tile([128, K2, 2, C], FP8, tag="wh", name="wh_s")

            # fp8 operand tiles with DoubleRow 2-plane interleave
            FS = [fspool.tile([128, 2, NL], FP8, tag=f"FS{k2}", name=f"FS{k2}")
                  for k2 in range(K2)]
            # Gt'[j, d] row-normalized style keys, j-major (DR over j-pairs)
            GT = [gtpool.tile([128, 2, C], FP8, tag=f"GT{j2}", name=f"GT{j2}")
                  for j2 in range(J2)]
            # [Hs | Hs^2] per j-block pair (full channel width, local tokens)
            H2 = [hpool.tile([128, 2, 2 * C], FP8, tag=f"H{j2}", name=f"H{j2}")
                  for j2 in range(J2)]
            # reduced B[d, c], B2[d, c]: bf16 from the exchange + fp8 DR copy
            Bbf = bpool.tile([128, K2, 2, C], BF16, tag="Bbf", name="Bbf")
            B2bf = bpool.tile([128, K2, 2, C], BF16, tag="B2bf", name="B2bf")
            B8 = bpool.tile([128, K2, 2, C], FP8, tag="B8", name="B8")
            B28 = bpool.tile([128, K2, 2, C], FP8, tag="B28", name="B28")

            # ---- small persistent tiles ----
            bf_sb = const.tile([128, KB], F32, tag="bf", name="bf")
            bg_row = const.tile([1, C], BF16, tag="bgr", name="bgr")
            ones_row = const.tile([1, 128], BF16, tag="ones1", name="ones1")
            # plane stride must be %16==0 for DoubleRow LdWeights
            ones_f8t = const.tile([128, 2, 32], FP8, tag="ones", name="ones")
            f_row = const.tile([1, NL], F32, tag="frow", name="frow")
            # u as DR stationary: [:, :, k2:k2+1] (plane stride 32)
            u2t = const.tile([128, 2, 32], FP8, tag="u2", name="u2")
            u_prow = const.tile([1, C], BF16, tag="upr", name="upr")
            u_brow = const.tile([1, C], BF16, tag="ubr", name="ubr")
            u_row = const.tile([1, C], FP8, tag="ur", name="ur")
            hrow = const.tile([1, C], BF16, tag="hrow", name="hrow")
            h2row = const.tile([1, C], BF16, tag="h2row", name="h2row")
            hs_sb = const.tile([128, CB], F32, tag="hs", name="hs")
            h2s_sb = const.tile([128, CB], F32, tag="h2s", name="h2s")
            bnpack = const.tile([128, 2 * CB], BF16, tag="bnp", name="bnp")
            cmean = const.tile([128, 16], F32, tag="cmean", name="cmean")
            cinv = const.tile([128, 16], F32, tag="cinv", name="cinv")
            negmc = const.tile([128, 16], F32, tag="negmc", name="negmc")
            eps_sb = const.tile([128, 16], F32, tag="eps", name="eps")

            # DRAM bounce buffers (exchange + row->partition transpose)
            u_d = drp.tile([1, C], FP8, tag="ud", name="ud")
            ccin = drp.tile([CCR, C], BF16, tag="ccin", name="ccin")
            ccout = drp.tile([2 * CCR, C], BF16, tag="ccout", name="ccout")

            # ---- DMA kickoff: few big loads, spread over queues ----
            nc.sync.dma_start(out=wgS, in_=wg[:, :, :, :])
            for k2 in range(K2):
                nc.sync.dma_start(out=SKT[k2], in_=sk[:, k2, :, :])
            nc.sync.dma_start(out=whS, in_=wh[:, :, :, :])
            for k2 in range(K2):
                nc.sync.dma_start(out=STT[k2], in_=st[:, k2, :, :])
            nc.sync.dma_start(out=wfS, in_=wf[:, :, :, :])
            for k2 in range(K2):
                nc.sync.dma_start(out=CKT[k2], in_=ck[:, k2, :, :])
            nc.scalar.dma_start(out=bg_row, in_=bgr[0:1, :])
            for cb in range(CB):
                nc.gpsimd.dma_start(out=CT[cb],
                                    in_=ct[cb * 128:(cb + 1) * 128, :])
            nc.gpsimd.dma_start(out=bf_sb, in_=bfb[:, :])
            nc.vector.memset(eps_sb, EPS)
            nc.vector.memset(ones_f8t, 1.0)
            nc.vector.memset(ones_row, 1.0)
            ones_f8 = ones_f8t[:, :, 0:1]

            # ---- Gt'[j, d] = (sk^T Wg + bg) / ||row|| (unit rows, fp8).
            #      Row norms alternate vector TTR / scalar Square-accum;
            #      sqrt+recip batched over 4 blocks; the fp8 scale-evac
            #      alternates scalar/vector. ----
            g2c4 = None
            gps = []
            for jb in range(QB):
                if jb % 4 == 0:
                    g2c4 = small.tile([128, 16], F32, tag="g2", name="g2")
                msl = slice(jb * 128, (jb + 1) * 128)
                ps = pp.tile([128, C], F32, tag="ps", name="ps")
                for k2 in range(K2):
                    nc.tensor.matmul(ps, SKT[k2][:, :, msl],
                                     wgS[:, k2, :, :], perf_mode=DR,
                                     start=(k2 == 0), stop=False)
                # + bg as rank-1 (ones_j x bg_d) into the same PSUM group
                nc.tensor.matmul(ps, ones_row, bg_row, start=False, stop=True)
                gjunk = gbp.tile([128, C], FP8, tag="gjunk", name="gjunk")
                q = jb % 4
                nc.scalar.activation(out=gjunk, in_=ps, func=AF.Square,
                                     accum_out=g2c4[:, q:q + 1])
                gps.append(ps)
                if q == 3:
                    gn4 = small.tile([128, 16], F32, tag="gn", name="gn")
                    nc.scalar.activation(out=gn4[:, 0:4], in_=g2c4[:, 0:4],
                                         func=AF.Sqrt)
                    ivg4 = small.tile([128, 16], F32, tag="ivg", name="ivg")
                    nc.vector.reciprocal_approx_fast(out=ivg4[:, 0:4],
                                                     in_=gn4[:, 0:4])
                    for w in range(4):
                        jw = jb - 3 + w
                        tgt = GT[jw // 2][:, jw % 2, :]
                        nc.vector.tensor_scalar_mul(tgt, gps[w],
                                                    ivg4[:, w:w + 1])
                    gps = []

            # ---- u partial = Gt'^T 1 over local tokens ----
            ups = pp.tile([1, C], F32, tag="ps", name="ups")
            for j2 in range(J2):
                nc.tensor.matmul(ups, ones_f8, GT[j2], perf_mode=DR,
                                 start=(j2 == 0), stop=(j2 == J2 - 1))
            nc.scalar.activation(out=u_prow, in_=ups, func=AF.Identity)
            nc.gpsimd.dma_start(out=ccin[2 * C + 2:2 * C + 3, :], in_=u_prow)

            # ---- Hs[j, c] = st^T WhT (full C, local tokens); [Hs | Hs^2] ----
            for jb in range(QB):
                msl = slice(jb * 128, (jb + 1) * 128)
                ps = pp.tile([128, C], F32, tag="ps", name="psh")
                for k2 in range(K2):
                    nc.tensor.matmul(ps, STT[k2][:, :, msl],
                                     whS[:, k2, :, :], perf_mode=DR,
                                     start=(k2 == 0), stop=(k2 == K2 - 1))
                hpl = H2[jb // 2][:, jb % 2, :]
                if jb % 2 == 0:
                    nc.scalar.activation(out=hpl[:, 0:C], in_=ps,
                                         func=AF.Identity)
                    nc.gpsimd.tensor_mul(hpl[:, C:2 * C], hpl[:, 0:C],
                                         hpl[:, 0:C])
                else:
                    nc.vector.tensor_copy(out=hpl[:, 0:C], in_=ps)
                    nc.scalar.activation(out=hpl[:, C:2 * C], in_=ps,
                                         func=AF.Square)

            # ---- B partial = Gt'^T Hs, B2 partial = Gt'^T Hs^2 ----
            for half in range(2):
                hsl = slice(half * C, (half + 1) * C)
                for db in range(CB):
                    ps = pp.tile([128, C], F32, tag="ps", name="psb")
                    for j2 in range(J2):
                        nc.tensor.matmul(
                            ps, GT[j2][:, :, db * 128:(db + 1) * 128],
                            H2[j2][:, :, hsl], perf_mode=DR,
                            start=(j2 == 0), stop=(j2 == J2 - 1))
                    stage = stg.tile([128, C], BF16, tag="stage", name="stage")
                    if db % 2 == 0:
                        nc.scalar.activation(out=stage, in_=ps,
                                             func=AF.Identity)
                    else:
                        nc.vector.tensor_copy(out=stage, in_=ps)
                    r0 = half * C + db * 128
                    nc.gpsimd.dma_start(out=ccin[r0:r0 + 128, :], in_=stage)

            # ---- content stats partials (own pixel half only) ----
            mtmp = small.tile([128, 16], F32, tag="bnm", name="bnm")
            vtmp = small.tile([128, 16], F32, tag="bnv", name="bnv")
            for cb in range(CB):
                stats = small.tile([128, PC, nc.vector.BN_STATS_DIM],
                                   F32, tag="bnstats", name="bnstats")
                for s_i in range(PC):
                    nc.vector.bn_stats(
                        out=stats[:, s_i, :],
                        in_=CT[cb][:, s_i * NC:(s_i + 1) * NC])
                mv = small.tile([128, nc.vector.BN_AGGR_DIM], F32,
                                tag="bnmv", name="bnmv")
                nc.vector.bn_aggr(out=mv, in_=stats)
                nc.gpsimd.tensor_copy(out=mtmp[:, cb:cb + 1], in_=mv[:, 0:1])
                nc.gpsimd.tensor_copy(out=vtmp[:, cb:cb + 1], in_=mv[:, 1:2])
            # pack 0.5*mean and 0.5*E[x^2] so the AllReduce sum is the merge
            etile = small.tile([128, 16], F32, tag="bne", name="bne")
            nc.gpsimd.tensor_mul(etile[:, 0:CB], mtmp[:, 0:CB], mtmp[:, 0:CB])
            nc.vector.tensor_add(etile[:, 0:CB], etile[:, 0:CB], vtmp[:, 0:CB])
            nc.vector.tensor_scalar_mul(bnpack[:, 0:CB], mtmp[:, 0:CB], 0.5)
            nc.vector.tensor_scalar_mul(bnpack[:, CB:2 * CB],
                                        etile[:, 0:CB], 0.5)
            nc.gpsimd.dma_start(
                out=ccin[2 * C + 3:2 * C + 5, :].rearrange(
                    "a (p c) -> (a p) c", p=64, c=2 * CB),
                in_=bnpack)

            # ---- hsum/h2sum partials: ones^T [Hs | Hs^2] ----
            for half, row in ((0, hrow), (1, h2row)):
                hps = pp.tile([1, C], F32, tag="ps", name="hps")
                for j2 in range(J2):
                    nc.tensor.matmul(hps, ones_f8,
                                     H2[j2][:, :, half * C:(half + 1) * C],
                                     perf_mode=DR,
                                     start=(j2 == 0), stop=(j2 == J2 - 1))
                nc.scalar.activation(out=row, in_=hps, func=AF.Identity)
            nc.gpsimd.dma_start(out=ccin[2 * C:2 * C + 1, :], in_=hrow)
            nc.gpsimd.dma_start(out=ccin[2 * C + 1:2 * C + 2, :], in_=h2row)

            # ---- pairwise AllReduce: one shot, F phase runs underneath ----
            nc.gpsimd.collective_compute(
                "AllGather", ALU.bypass, replica_groups=GROUPS,
                ins=[ccin.opt()], outs=[ccout.opt()])

            # ---- F = Wf ck + bf (channel-major [C, NL]) + column norms;
            #      FS = F / f (unit-norm columns).  The PE/scalar part runs
            #      during the collective; the fbc broadcasts and FS scaling
            #      sit after it on their queues. ----
            for pc in range(PC):
                psl = slice(pc * NC, (pc + 1) * NC)
                fsq = [etmp.tile([128, 2, NC], FP8, tag=f"fsq{k2}", name="fsq")
                       for k2 in range(K2)]
                f2ps = pp.tile([1, NC], F32, tag="ps", name="f2ps")
                fps = []
                for ob in range(CB):
                    ps = pp.tile([128, NC], F32, tag="ps", name="fps")
                    for k2 in range(K2):
                        nc.tensor.matmul(ps,
                                         wfS[:, k2, :, ob * 128:(ob + 1) * 128],
                                         CKT[k2][:, :, psl], perf_mode=DR,
                                         start=(k2 == 0), stop=(k2 == K2 - 1))
                    fps.append(ps)
                    # norm contribution straight from PSUM: (ps + bf)^2
                    nc.scalar.activation(out=fsq[ob // 2][:, ob % 2, :], in_=ps,
                                         func=AF.Square,
                                         bias=bf_sb[:, ob:ob + 1])
                for k2 in range(K2):
                    nc.tensor.matmul(f2ps, ones_f8, fsq[k2], perf_mode=DR,
                                     start=(k2 == 0), stop=(k2 == K2 - 1))
                nc.scalar.activation(out=f_row[0:1, psl], in_=f2ps,
                                     func=AF.Sqrt)
                frec = etmp.tile([1, NC], F32, tag="frec", name="frec")
                nc.vector.reciprocal_approx_fast(out=frec, in_=f_row[0:1, psl])
                fbc = etmp.tile([128, NC], F32, tag="fbc", name="fbc")
                nc.gpsimd.partition_broadcast(fbc, frec)
                # fused evac: FS = (ps + bf) * (1/f) -> fp8, unit-norm columns
                for ob in range(CB):
                    nc.vector.scalar_tensor_tensor(
                        FS[ob // 2][:, ob % 2, psl], fps[ob],
                        bf_sb[:, ob:ob + 1], fbc,
                        op0=ALU.add, op1=ALU.mult)

            # ---- exchange readback: both slots + local adds
            #      (sync: B first — it gates the PE) ----
            Bh = [bpool.tile([128, K2, 2, C], BF16, tag=f"Bh{s}",
                             name=f"Bh{s}") for s in range(2)]
            B2h = [bpool.tile([128, K2, 2, C], BF16, tag=f"B2h{s}",
                              name=f"B2h{s}") for s in range(2)]
            # u first: it gates the den chains (tiny transfers)
            ub = [const.tile([1, C], BF16, tag=f"ub{s}", name=f"ub{s}")
                  for s in range(2)]
            for s in range(2):
                r0 = s * CCR + 2 * C + 2
                nc.sync.dma_start(out=ub[s], in_=ccout[r0:r0 + 1, :])
            nc.vector.tensor_add(u_brow, ub[0], ub[1])
            nc.vector.tensor_copy(out=u_row, in_=u_brow)
            nc.sync.dma_start(out=u_d, in_=u_row)
            for k2 in range(K2):
                nc.sync.dma_start(
                    out=u2t[:, :, k2:k2 + 1],
                    in_=u_d[0:1, k2 * 256:(k2 + 1) * 256].rearrange(
                        "p (two r) -> (p r) two", two=2, r=128))
            for s in range(2):
                r0 = s * CCR
                nc.sync.dma_start(
                    out=Bh[s],
                    in_=ccout[r0:r0 + C, :].rearrange(
                        "(k2 pl p) c -> p k2 pl c", k2=K2, pl=2, p=128))
            nc.vector.tensor_add(Bbf, Bh[0], Bh[1])
            nc.scalar.activation(out=B8, in_=Bbf, func=AF.Identity)
            for s in range(2):
                r0 = s * CCR + C
                nc.scalar.dma_start(
                    out=B2h[s],
                    in_=ccout[r0:r0 + C, :].rearrange(
                        "(k2 pl p) c -> p k2 pl c", k2=K2, pl=2, p=128))
            nc.vector.tensor_add(B2bf, B2h[0], B2h[1])
            nc.scalar.activation(out=B28, in_=B2bf, func=AF.Identity)
            # per-channel rows -> partition-major scalars (gpsimd queue)
            hsh = [const.tile([128, CB], BF16, tag=f"hsh{s}", name=f"hsh{s}")
                   for s in range(2)]
            h2sh = [const.tile([128, CB], BF16, tag=f"h2sh{s}",
                               name=f"h2sh{s}") for s in range(2)]
            bnh = [const.tile([128, 2 * CB], BF16, tag=f"bnh{s}",
                              name=f"bnh{s}") for s in range(2)]
            for s in range(2):
                r0 = s * CCR + 2 * C
                nc.gpsimd.dma_start(
                    out=hsh[s], in_=ccout[r0:r0 + 1, :].rearrange(
                        "one (cb p) -> (one p) cb", cb=CB, p=128))
                nc.gpsimd.dma_start(
                    out=h2sh[s], in_=ccout[r0 + 1:r0 + 2, :].rearrange(
                        "one (cb p) -> (one p) cb", cb=CB, p=128))
                nc.gpsimd.dma_start(
                    out=bnh[s], in_=ccout[r0 + 3:r0 + 5, :].rearrange(
                        "a (p c) -> (a p) c", p=64, c=2 * CB))
            nc.vector.tensor_add(hs_sb, hsh[0], hsh[1])
            nc.vector.tensor_add(h2s_sb, h2sh[0], h2sh[1])
            bn_f = small.tile([128, 2 * CB], F32, tag="bnf", name="bnf")
            nc.vector.tensor_add(bn_f, bnh[0], bnh[1])

            # ---- den chains: their latency overlaps the readback ----
            Mc = float(N)
            ivbcs = []
            for pc in range(PC):
                psl = slice(pc * NC, (pc + 1) * NC)
                dps = pp.tile([1, NC], F32, tag="ps", name="dps")
                for k2 in range(K2):
                    nc.tensor.matmul(dps, u2t[:, :, k2:k2 + 1],
                                     FS[k2][:, :, psl], perf_mode=DR,
                                     start=(k2 == 0), stop=(k2 == K2 - 1))
                den = ivp.tile([1, NC], F32, tag=f"den{pc}", name="den")
                nc.vector.tensor_scalar_add(den, dps, Mc + EPS)
                ivd = ivp.tile([1, NC], F32, tag=f"ivd{pc}", name="ivd")
                nc.vector.reciprocal_approx_fast(out=ivd, in_=den)
                ivbc = ivp.tile([128, NC], F32, tag=f"ivbc{pc}", name="ivbc")
                nc.gpsimd.partition_broadcast(ivbc, ivd)
                ivbcs.append(ivbc)

            # ---- merge bn stats: mean/var over the full image ----
            mm4 = small.tile([128, 16], F32, tag="bmm", name="bmm")
            nc.gpsimd.tensor_mul(mm4[:, 0:CB], bn_f[:, 0:CB], bn_f[:, 0:CB])
            varb = small.tile([128, 16], F32, tag="bvar", name="bvar")
            nc.vector.tensor_sub(varb[:, 0:CB], bn_f[:, CB:2 * CB],
                                 mm4[:, 0:CB])
            nc.vector.tensor_copy(out=cmean[:, 0:CB], in_=bn_f[:, 0:CB])
            cstd = small.tile([128, 16], F32, tag="cstd", name="cstd")
            nc.scalar.activation(out=cstd[:, 0:CB], in_=varb[:, 0:CB],
                                 func=AF.Sqrt, bias=eps_sb[:, 0:1],
                                 scale=float(NT) / (NT - 1))
            nc.vector.reciprocal_approx_fast(out=cinv[:, 0:CB],
                                             in_=cstd[:, 0:CB])
            nc.vector.tensor_mul(negmc[:, 0:CB], cmean[:, 0:CB],
                                 cinv[:, 0:CB])
            nc.vector.tensor_scalar_mul(negmc[:, 0:CB], negmc[:, 0:CB], -1.0)
            # normalize content in place: CT <- (ct - cmean) / cstd
            for cb in range(CB):
                nc.scalar.activation(out=CT[cb], in_=CT[cb], func=AF.Identity,
                                     scale=cinv[:, cb:cb + 1],
                                     bias=negmc[:, cb:cb + 1])

            # ---- main loop: mean/sq from reduced B, B2 + AdaIN combine.
            #      PSUM evacs are per-bank STTs into halves of [128, 1024]
            #      tiles; the rest of the combine runs 1024-wide to halve
            #      per-op overhead.  (+bh is applied on the host.) ----
            for pw in range(PW):
                wsl = slice(pw * 2 * NC, (pw + 1) * 2 * NC)
                for cb in range(CB):
                    csl = slice(cb * 128, (cb + 1) * 128)
                    mean_t = cmb.tile([128, 2 * NC], BF16, tag="mean",
                                      name="mean")
                    sqs_t = cmb.tile([128, 2 * NC], BF16, tag="sqs",
                                     name="sqs")
                    for half in range(2):
                        pc = pw * 2 + half
                        psl = slice(pc * NC, (pc + 1) * NC)
                        dsl = slice(half * NC, (half + 1) * NC)
                        psm = pp.tile([128, NC], F32, tag="ps", name="psm")
                        for k2 in range(K2):
                            nc.tensor.matmul(psm, B8[:, k2, :, csl],
                                             FS[k2][:, :, psl], perf_mode=DR,
                                             start=(k2 == 0),
                                             stop=(k2 == K2 - 1))
                        pss = pp.tile([128, NC], F32, tag="ps", name="pss")
                        for k2 in range(K2):
                            nc.tensor.matmul(pss, B28[:, k2, :, csl],
                                             FS[k2][:, :, psl], perf_mode=DR,
                                             start=(k2 == 0),
                                             stop=(k2 == K2 - 1))
                        # mean = (hsum + B^T F') / den ; sq likewise
                        nc.vector.scalar_tensor_tensor(
                            mean_t[:, dsl], psm, hs_sb[:, cb:cb + 1],
                            ivbcs[pc], op0=ALU.add, op1=ALU.mult)
                        nc.vector.scalar_tensor_tensor(
                            sqs_t[:, dsl], pss, h2s_sb[:, cb:cb + 1],
                            ivbcs[pc], op0=ALU.add, op1=ALU.mult)
                    m2_t = cmb.tile([128, 2 * NC], BF16, tag="m2", name="m2")
                    nc.scalar.activation(out=m2_t, in_=mean_t, func=AF.Square)
                    nc.gpsimd.tensor_sub(sqs_t, sqs_t, m2_t)
                    sd_t = cmb.tile([128, 2 * NC], BF16, tag="sd", name="sd")
                    nc.scalar.activation(out=sd_t, in_=sqs_t, func=AF.Sqrt)
                    out_t = op.tile([128, 2 * NC], BF16, tag="out",
                                    name="out_t")
                    nc.vector.tensor_mul(out_t, sd_t, CT[cb][:, wsl])
                    nc.vector.tensor_add(out_t, out_t, mean_t)
                    nc.sync.dma_start(out=out[csl, wsl], in_=out_t)

    nc.finalize()
    return nc


_NC_CACHE = {}


def _get_nc(C, N, NL):
    key = (C, N, NL)
    if key not in _NC_CACHE:
        _NC_CACHE[key] = build_nc(C, N, NL)
    return _NC_CACHE[key]


def make_in_maps(content, style, content_key, style_key, Wf, bf, Wg, bg, Wh, bh):
    """Shard full inputs into 8 per-core input maps."""
    B, C, H, W = content.shape
    NP = H * W
    NL = NP // 2
    KB = C // 128

    def prep(x):
        return np.ascontiguousarray(x, dtype=np.float32)

    def prep16(x):
        return np.ascontiguousarray(np.asarray(x).astype(ml_dtypes.bfloat16))

    def prep8i(x):  # [C, n] -> [128, KB//2, 2, n] fp8 DoubleRow interleave
        Cd, n = x.shape
        k2 = Cd // 256
        return np.ascontiguousarray(
            np.asarray(x).reshape(k2, 2, 128, n).transpose(2, 0, 1, 3)
        ).astype(ml_dtypes.float8_e4m3)

    wfT = prep8i(np.asarray(Wf).T)
    wgT = prep8i(np.asarray(Wg).T)
    whT = prep8i(np.asarray(Wh).T)
    bfb = prep(np.asarray(bf).reshape(KB, 128).T)
    bgrr = prep16(np.asarray(bg).reshape(1, C))

    in_maps = []
    for core in range(8):
        b, h = core // 2, core % 2
        hsl = slice(h * NL, (h + 1) * NL)
        in_maps.append({
            "ck": prep8i(np.asarray(content_key[b]).reshape(C, NP)[:, hsl]),
            "sk": prep8i(np.asarray(style_key[b]).reshape(C, NP)[:, hsl]),
            "st": prep8i(np.asarray(style[b]).reshape(C, NP)[:, hsl]),
            "ct": prep16(np.asarray(content[b]).reshape(C, NP)[:, hsl]),
            "wf": wfT, "wg": wgT, "wh": whT,
            "bfb": bfb, "bgr": bgrr,
        })
    return in_maps


def kernel(content, style, content_key, style_key, Wf, bf, Wg, bg, Wh, bh,
           _trace=False):
    B, C, H, W = content.shape
    NP = H * W
    NL = NP // 2
    nc = _get_nc(C, NP, NL)
    in_maps = make_in_maps(content, style, content_key, style_key,
                           Wf, bf, Wg, bg, Wh, bh)
    res = run_bass_kernel_spmd(nc, in_maps, core_ids=list(range(8)), trace=_trace)
    out = np.empty((B, C, NP), dtype=np.float32)
    for core in range(8):
        b, h = core // 2, core % 2
        out[b, :, h * NL:(h + 1) * NL] = res.results[core]["out"]
    # the conv bias bh shifts mean only (it cancels inside std): add it here
    out += np.asarray(bh, dtype=np.float32)[None, :, None]
    if _trace:
        kernel.last_results = res
    return out.reshape(B, C, H, W)


# revision 18
# speedup vs baseline: 1.0906x; 1.0906x over previous
"""Trainium2 Bass kernel for AttnAdaINCos (cosine-attention AdaIN style transfer).

Sharding: 8 cores = 4 batches x 2 halves. Within a pair the content-pixel
axis is split (sequence parallel) AND the style-token axis is split: core h
computes Gt'/Hs/B/B2/u/hsum partials over its half of the style tokens (the
halves arrive via the per-core input map, so the program is identical on
both cores) plus content stats for its pixel half.  One pairwise bf16
AllGather (DRAM bounce, ~15-20us fixed cost) exchanges B/B2/u/hsum/h2sum/bn
partials in a single shot; cheap local adds merge the two slots.

Key identity: cos in [-1, 1] by Cauchy-Schwarz, so the reference's
S = relu(cos + 1) = 1 + cos is LINEAR in cos — the attention collapses
associatively. With unit-normalized key features F'[d,p] (content) and
Gt'[j,d] (style, row-normalized, transposed) and style values Hs[j,c]:

  mean_num[c,p] = sum_j (1+cos_jp) Hs[j,c] = hsum[c] + (B^T F')[c,p],
      B[d,c] = sum_j Gt'[j,d] Hs[j,c]     [C x C, reduced across the pair]
  sq_num uses B2 = Gt'^T Hs^2 and h2sum
  den_p = M + u^T F'_p + EPS,  u[d] = sum_j Gt'[j,d]

  mean = mean_num/den + bh    (Hs bias cancels in std, shifts mean)
  std  = sqrt(sq_num/den - (mean-bh)^2)
  out = std * (ct - cmean)/cstd + mean    (cmean/cstd over the full image,
                                           merged from per-half bn partials)

Dtypes: all matrix products run fp8e4 DoubleRow on [128, K/256, 2, n]
interleaved operands with fp32 PSUM; the exchange runs bf16; combine fp32.
The PE p-state ramp rewards uninterrupted matmul streams, so inputs load
with few big DMAs up front and evacuations alternate scalar/vector/gpsimd.
The gpsimd queue is FIFO and the collective blocks it, so everything gpsimd
that is needed while the collective is in flight is emitted after it.
"""

import sys

if "/opt/trn_rl_repo" not in sys.path:
    sys.path.insert(0, "/opt/trn_rl_repo")

from contextlib import ExitStack

import numpy as np

import ml_dtypes

import concourse.bass as bass
import concourse.tile as tile
from concourse import bacc, mybir
from concourse.bass_utils import run_bass_kernel_spmd

F32 = mybir.dt.float32
BF16 = mybir.dt.bfloat16
FP8 = mybir.dt.float8e4
DR = mybir.MatmulPerfMode.DoubleRow
AF = mybir.ActivationFunctionType
ALU = mybir.AluOpType
PS = bass.MemorySpace.PSUM
EPS = 1e-5
NC = 512  # free-dim chunk size (one PSUM bank of fp32)

GROUPS = [[0, 1], [2, 3], [4, 5], [6, 7]]


def build_nc(C=512, N=4096, NL=2048):
    """Build the single SPMD program (identical on all cores)."""
    KB = C // 128     # channel 128-blocks (contraction blocks)
    CB = C // 128     # output-channel 128-blocks
    N2 = N // 2       # style tokens processed locally
    QB = N2 // 128    # local style-token 128-blocks (j-blocks)
    J2 = QB // 2      # local style-token 256-pair tiles
    PC = NL // NC     # local-pixel 512-chunks
    PW = PC // 2      # local-pixel 1024-wide combine groups
    NT = 2 * NL       # full-image pixel count (for content stats)
    K2 = KB // 2
    CCR = C + C + 5   # exchange rows: B, B2, hsum, h2sum, u, bn(2)

    nc = bacc.Bacc("TRN2", target_bir_lowering=False, num_devices=8)

    ck = nc.dram_tensor("ck", [128, K2, 2, NL], FP8, kind="ExternalInput")
    sk = nc.dram_tensor("sk", [128, K2, 2, N2], FP8, kind="ExternalInput")
    st = nc.dram_tensor("st", [128, K2, 2, N2], FP8, kind="ExternalInput")
    ct = nc.dram_tensor("ct", [C, NL], BF16, kind="ExternalInput")
    wf = nc.dram_tensor("wf", [128, K2, 2, C], FP8, kind="ExternalInput")
    wg = nc.dram_tensor("wg", [128, K2, 2, C], FP8, kind="ExternalInput")
    wh = nc.dram_tensor("wh", [128, K2, 2, C], FP8, kind="ExternalInput")
    bfb = nc.dram_tensor("bfb", [128, KB], F32, kind="ExternalInput")
    bgr = nc.dram_tensor("bgr", [1, C], BF16, kind="ExternalInput")
    out = nc.dram_tensor("out", [C, NL], BF16, kind="ExternalOutput")

    with tile.TileContext(nc) as tc:
        with ExitStack() as stk:
            # big 64B-multiple matmul operands first (keeps them aligned)
            rsk = stk.enter_context(tc.tile_pool(name="rsk", bufs=1))
            rct = stk.enter_context(tc.tile_pool(name="rct", bufs=1))
            resw = stk.enter_context(tc.tile_pool(name="resw", bufs=1))
            gtpool = stk.enter_context(tc.tile_pool(name="gtpool", bufs=1))
            hpool = stk.enter_context(tc.tile_pool(name="hpool", bufs=1))
            fspool = stk.enter_context(tc.tile_pool(name="fspool", bufs=1))
            bpool = stk.enter_context(tc.tile_pool(name="bpool", bufs=1))
            gbp = stk.enter_context(tc.tile_pool(name="gbp", bufs=2))
            stg = stk.enter_context(tc.tile_pool(name="stg", bufs=4))
            etmp = stk.enter_context(tc.tile_pool(name="etmp", bufs=4))
            ivp = stk.enter_context(tc.tile_pool(name="ivp", bufs=1))
            cmb = stk.enter_context(tc.tile_pool(name="cmb", bufs=2))
            op = stk.enter_context(tc.tile_pool(name="op", bufs=3))
            pp = stk.enter_context(tc.tile_pool(name="pp", bufs=8, space=PS))
            # odd-sized tiles last
            const = stk.enter_context(tc.tile_pool(name="const", bufs=1))
            small = stk.enter_context(tc.tile_pool(name="small", bufs=4))
            drp = stk.enter_context(
                tc.tile_pool(name="drp", bufs=1, space=bass.MemorySpace.DRAM))

            # ---- resident input tiles ----
            SKT = [rsk.tile([128, 2, N2], FP8, tag=f"SK{k2}", name=f"SK{k2}")
                   for k2 in range(K2)]
            STT = [rsk.tile([128, 2, N2], FP8, tag=f"ST{k2}", name=f"ST{k2}")
                   for k2 in range(K2)]
            CKT = [rsk.tile([128, 2, NL], FP8, tag=f"CK{k2}", name=f"CK{k2}")
                   for k2 in range(K2)]
            CT = [rct.tile([128, NL], BF16, tag=f"CT{cb}", name=f"CT{cb}")
                  for cb in range(CB)]
            wfS = resw.tile([128, K2, 2, C], FP8, tag="wf", name="wf_s")
            wgS = resw.tile([128, K2, 2, C], FP8, tag="wg", name="wg_s")
            whS = resw.tile([128, K2, 2, C], FP8, tag="wh", name="wh_s")

            # fp8 operand tiles with DoubleRow 2-plane interleave
            FS = [fspool.tile([128, 2, NL], FP8, tag=f"FS{k2}", name=f"FS{k2}")
                  for k2 in range(K2)]
            # Gt'[j, d] row-normalized style keys, j-major (DR over j-pairs)
            GT = [gtpool.tile([128, 2, C], FP8, tag=f"GT{j2}", name=f"GT{j2}")
                  for j2 in range(J2)]
            # [Hs | Hs^2] per j-block pair (full channel width, local tokens)
            H2 = [hpool.tile([128, 2, 2 * C], FP8, tag=f"H{j2}", name=f"H{j2}")
                  for j2 in range(J2)]
            # reduced B[d, c], B2[d, c]: bf16 from the exchange + fp8 DR copy
            Bbf = bpool.tile([128, K2, 2, C], BF16, tag="Bbf", name="Bbf")
            B2bf = bpool.tile([128, K2, 2, C], BF16, tag="B2bf", name="B2bf")
            B8 = bpool.tile([128, K2, 2, C], FP8, tag="B8", name="B8")
            B28 = bpool.tile([128, K2, 2, C], FP8, tag="B28", name="B28")

            # ---- small persistent tiles ----
            bf_sb = const.tile([128, KB], F32, tag="bf", name="bf")
            bg_row = const.tile([1, C], BF16, tag="bgr", name="bgr")
            ones_row = const.tile([1, 128], BF16, tag="ones1", name="ones1")
            # plane stride must be %16==0 for DoubleRow LdWeights
            ones_f8t = const.tile([128, 2, 32], FP8, tag="ones", name="ones")
            f_row = const.tile([1, NL], F32, tag="frow", name="frow")
            # u as DR stationary: [:, :, k2:k2+1] (plane stride 32)
            u2t = const.tile([128, 2, 32], FP8, tag="u2", name="u2")
            u_prow = const.tile([1, C], BF16, tag="upr", name="upr")
            u_brow = const.tile([1, C], BF16, tag="ubr", name="ubr")
            u_row = const.tile([1, C], FP8, tag="ur", name="ur")
            hrow = const.tile([1, C], BF16, tag="hrow", name="hrow")
            h2row = const.tile([1, C], BF16, tag="h2row", name="h2row")
            hs_sb = const.tile([128, CB], F32, tag="hs", name="hs")
            h2s_sb = const.tile([128, CB], F32, tag="h2s", name="h2s")
            bnpack = const.tile([128, 2 * CB], BF16, tag="bnp", name="bnp")
            cmean = const.tile([128, 16], F32, tag="cmean", name="cmean")
            cinv = const.tile([128, 16], F32, tag="cinv", name="cinv")
            negmc = const.tile([128, 16], F32, tag="negmc", name="negmc")
            eps_sb = const.tile([128, 16], F32, tag="eps", name="eps")

            # DRAM bounce buffers (exchange + row->partition transpose)
            u_d = drp.tile([1, C], FP8, tag="ud", name="ud")
            ccin = drp.tile([CCR, C], BF16, tag="ccin", name="ccin")
            ccout = drp.tile([2 * CCR, C], BF16, tag="ccout", name="ccout")

            # ---- DMA kickoff: few big loads, spread over queues ----
            nc.sync.dma_start(out=wgS, in_=wg[:, :, :, :])
            for k2 in range(K2):
                nc.sync.dma_start(out=SKT[k2], in_=sk[:, k2, :, :])
            nc.sync.dma_start(out=whS, in_=wh[:, :, :, :])
            for k2 in range(K2):
                nc.sync.dma_start(out=STT[k2], in_=st[:, k2, :, :])
            nc.sync.dma_start(out=wfS, in_=wf[:, :, :, :])
            for k2 in range(K2):
                nc.sync.dma_start(out=CKT[k2], in_=ck[:, k2, :, :])
            nc.scalar.dma_start(out=bg_row, in_=bgr[0:1, :])
            for cb in range(CB):
                nc.gpsimd.dma_start(out=CT[cb],
                                    in_=ct[cb * 128:(cb + 1) * 128, :])
            nc.gpsimd.dma_start(out=bf_sb, in_=bfb[:, :])
            nc.vector.memset(eps_sb, EPS)
            nc.vector.memset(ones_f8t, 1.0)
            nc.vector.memset(ones_row, 1.0)
            ones_f8 = ones_f8t[:, :, 0:1]

            # ---- Gt'[j, d] = (sk^T Wg + bg) / ||row|| (unit rows, fp8).
            #      Row norms alternate vector TTR / scalar Square-accum;
            #      sqrt+recip batched over 4 blocks; the fp8 scale-evac
            #      alternates scalar/vector. ----
            g2c4 = None
            gps = []
            for jb in range(QB):
                if jb % 4 == 0:
                    g2c4 = small.tile([128, 16], F32, tag="g2", name="g2")
                msl = slice(jb * 128, (jb + 1) * 128)
                ps = pp.tile([128, C], F32, tag="ps", name="ps")
                for k2 in range(K2):
                    nc.tensor.matmul(ps, SKT[k2][:, :, msl],
                                     wgS[:, k2, :, :], perf_mode=DR,
                                     start=(k2 == 0), stop=False)
                # + bg as rank-1 (ones_j x bg_d) into the same PSUM group
                nc.tensor.matmul(ps, ones_row, bg_row, start=False, stop=True)
                gjunk = gbp.tile([128, C], FP8, tag="gjunk", name="gjunk")
                q = jb % 4
                nc.scalar.activation(out=gjunk, in_=ps, func=AF.Square,
                                     accum_out=g2c4[:, q:q + 1])
                gps.append(ps)
                if q == 3:
                    gn4 = small.tile([128, 16], F32, tag="gn", name="gn")
                    nc.scalar.activation(out=gn4[:, 0:4], in_=g2c4[:, 0:4],
                                         func=AF.Sqrt)
                    ivg4 = small.tile([128, 16], F32, tag="ivg", name="ivg")
                    nc.vector.reciprocal_approx_fast(out=ivg4[:, 0:4],
                                                     in_=gn4[:, 0:4])
                    for w in range(4):
                        jw = jb - 3 + w
                        tgt = GT[jw // 2][:, jw % 2, :]
                        nc.vector.tensor_scalar_mul(tgt, gps[w],
                                                    ivg4[:, w:w + 1])
                    gps = []

            # ---- u partial = Gt'^T 1 over local tokens ----
            ups = pp.tile([1, C], F32, tag="ps", name="ups")
            for j2 in range(J2):
                nc.tensor.matmul(ups, ones_f8, GT[j2], perf_mode=DR,
                                 start=(j2 == 0), stop=(j2 == J2 - 1))
            nc.scalar.activation(out=u_prow, in_=ups, func=AF.Identity)
            nc.gpsimd.dma_start(out=ccin[2 * C + 2:2 * C + 3, :], in_=u_prow)

            # ---- Hs[j, c] = st^T WhT (full C, local tokens); [Hs | Hs^2] ----
            for jb in range(QB):
                msl = slice(jb * 128, (jb + 1) * 128)
                ps = pp.tile([128, C], F32, tag="ps", name="psh")
                for k2 in range(K2):
                    nc.tensor.matmul(ps, STT[k2][:, :, msl],
                                     whS[:, k2, :, :], perf_mode=DR,
                                     start=(k2 == 0), stop=(k2 == K2 - 1))
                hpl = H2[jb // 2][:, jb % 2, :]
                if jb % 2 == 0:
                    nc.scalar.activation(out=hpl[:, 0:C], in_=ps,
                                         func=AF.Identity)
                    nc.gpsimd.tensor_mul(hpl[:, C:2 * C], hpl[:, 0:C],
                                         hpl[:, 0:C])
                else:
                    nc.vector.tensor_copy(out=hpl[:, 0:C], in_=ps)
                    nc.scalar.activation(out=hpl[:, C:2 * C], in_=ps,
                                         func=AF.Square)

            # ---- content stats partials (own pixel half only) ----
            mtmp = small.tile([128, 16], F32, tag="bnm", name="bnm")
            vtmp = small.tile([128, 16], F32, tag="bnv", name="bnv")
            for cb in range(CB):
                stats = small.tile([128, PC, nc.vector.BN_STATS_DIM],
                                   F32, tag="bnstats", name="bnstats")
                for s_i in range(PC):
                    nc.vector.bn_stats(
                        out=stats[:, s_i, :],
                        in_=CT[cb][:, s_i * NC:(s_i + 1) * NC])
                mv = small.tile([128, nc.vector.BN_AGGR_DIM], F32,
                                tag="bnmv", name="bnmv")
                nc.vector.bn_aggr(out=mv, in_=stats)
                nc.gpsimd.tensor_copy(out=mtmp[:, cb:cb + 1], in_=mv[:, 0:1])
                nc.gpsimd.tensor_copy(out=vtmp[:, cb:cb + 1], in_=mv[:, 1:2])
            # pack 0.5*mean and 0.5*E[x^2] so the AllReduce sum is the merge
            etile = small.tile([128, 16], F32, tag="bne", name="bne")
            nc.gpsimd.tensor_mul(etile[:, 0:CB], mtmp[:, 0:CB], mtmp[:, 0:CB])
            nc.vector.tensor_add(etile[:, 0:CB], etile[:, 0:CB], vtmp[:, 0:CB])
            nc.vector.tensor_scalar_mul(bnpack[:, 0:CB], mtmp[:, 0:CB], 0.5)
            nc.vector.tensor_scalar_mul(bnpack[:, CB:2 * CB],
                                        etile[:, 0:CB], 0.5)
            nc.gpsimd.dma_start(
                out=ccin[2 * C + 3:2 * C + 5, :].rearrange(
                    "a (p c) -> (a p) c", p=64, c=2 * CB),
                in_=bnpack)

            # ---- B partial = Gt'^T Hs, B2 partial = Gt'^T Hs^2 ----
            for half in range(2):
                hsl = slice(half * C, (half + 1) * C)
                for db in range(CB):
                    ps = pp.tile([128, C], F32, tag="ps", name="psb")
                    for j2 in range(J2):
                        nc.tensor.matmul(
                            ps, GT[j2][:, :, db * 128:(db + 1) * 128],
                            H2[j2][:, :, hsl], perf_mode=DR,
                            start=(j2 == 0), stop=(j2 == J2 - 1))
                    stage = stg.tile([128, C], BF16, tag="stage", name="stage")
                    if db % 2 == 0:
                        nc.scalar.activation(out=stage, in_=ps,
                                             func=AF.Identity)
                    else:
                        nc.vector.tensor_copy(out=stage, in_=ps)
                    r0 = half * C + db * 128
                    nc.gpsimd.dma_start(out=ccin[r0:r0 + 128, :], in_=stage)

            # ---- hsum/h2sum partials: ones^T [Hs | Hs^2] ----
            for half, row in ((0, hrow), (1, h2row)):
                hps = pp.tile([1, C], F32, tag="ps", name="hps")
                for j2 in range(J2):
                    nc.tensor.matmul(hps, ones_f8,
                                     H2[j2][:, :, half * C:(half + 1) * C],
                                     perf_mode=DR,
                                     start=(j2 == 0), stop=(j2 == J2 - 1))
                nc.scalar.activation(out=row, in_=hps, func=AF.Identity)
            nc.gpsimd.dma_start(out=ccin[2 * C:2 * C + 1, :], in_=hrow)
            nc.gpsimd.dma_start(out=ccin[2 * C + 1:2 * C + 2, :], in_=h2row)

            # ---- pairwise AllReduce: one shot, F phase runs underneath ----
            nc.gpsimd.collective_compute(
                "AllGather", ALU.bypass, replica_groups=GROUPS,
                ins=[ccin.opt()], outs=[ccout.opt()])

            # ---- F = Wf ck + bf (channel-major [C, NL]) + column norms;
            #      FS = F / f (unit-norm columns).  The PE/scalar part runs
            #      during the collective; the fbc broadcasts and FS scaling
            #      sit after it on their queues. ----
            for pc in range(PC):
                psl = slice(pc * NC, (pc + 1) * NC)
                fsq = [etmp.tile([128, 2, NC], FP8, tag=f"fsq{k2}", name="fsq")
                       for k2 in range(K2)]
                f2ps = pp.tile([1, NC], F32, tag="ps", name="f2ps")
                fps = []
                for ob in range(CB):
                    ps = pp.tile([128, NC], F32, tag="ps", name="fps")
                    for k2 in range(K2):
                        nc.tensor.matmul(ps,
                                         wfS[:, k2, :, ob * 128:(ob + 1) * 128],
                                         CKT[k2][:, :, psl], perf_mode=DR,
                                         start=(k2 == 0), stop=(k2 == K2 - 1))
                    fps.append(ps)
                    # norm contribution straight from PSUM: (ps + bf)^2
                    nc.scalar.activation(out=fsq[ob // 2][:, ob % 2, :], in_=ps,
                                         func=AF.Square,
                                         bias=bf_sb[:, ob:ob + 1])
                for k2 in range(K2):
                    nc.tensor.matmul(f2ps, ones_f8, fsq[k2], perf_mode=DR,
                                     start=(k2 == 0), stop=(k2 == K2 - 1))
                nc.scalar.activation(out=f_row[0:1, psl], in_=f2ps,
                                     func=AF.Sqrt)
                frec = etmp.tile([1, NC], F32, tag="frec", name="frec")
                nc.vector.reciprocal_approx_fast(out=frec, in_=f_row[0:1, psl])
                frecb = etmp.tile([1, NC], BF16, tag="frecb", name="frecb")
                nc.vector.tensor_copy(out=frecb, in_=frec)
                # broadcast 1/f across partitions on the PE (the gpsimd queue
                # is blocked by the in-flight collective)
                fbp = pp.tile([128, NC], F32, tag="ps", name="fbp")
                nc.tensor.matmul(fbp, ones_row, frecb, start=True, stop=True)
                fbc = etmp.tile([128, NC], F32, tag="fbc", name="fbc")
                nc.vector.tensor_copy(out=fbc, in_=fbp)
                # fused evac: FS = (ps + bf) * (1/f) -> fp8, unit-norm columns
                for ob in range(CB):
                    nc.vector.scalar_tensor_tensor(
                        FS[ob // 2][:, ob % 2, psl], fps[ob],
                        bf_sb[:, ob:ob + 1], fbc,
                        op0=ALU.add, op1=ALU.mult)

            # ---- exchange readback: both slots + local adds
            #      (sync: B first — it gates the PE) ----
            Bh = [bpool.tile([128, K2, 2, C], BF16, tag=f"Bh{s}",
                             name=f"Bh{s}") for s in range(2)]
            B2h = [bpool.tile([128, K2, 2, C], BF16, tag=f"B2h{s}",
                              name=f"B2h{s}") for s in range(2)]
            # u first: it gates the den chains (tiny transfers)
            ub = [const.tile([1, C], BF16, tag=f"ub{s}", name=f"ub{s}")
                  for s in range(2)]
            for s in range(2):
                r0 = s * CCR + 2 * C + 2
                nc.sync.dma_start(out=ub[s], in_=ccout[r0:r0 + 1, :])
            nc.vector.tensor_add(u_brow, ub[0], ub[1])
            nc.vector.tensor_copy(out=u_row, in_=u_brow)
            nc.sync.dma_start(out=u_d, in_=u_row)
            for k2 in range(K2):
                nc.sync.dma_start(
                    out=u2t[:, :, k2:k2 + 1],
                    in_=u_d[0:1, k2 * 256:(k2 + 1) * 256].rearrange(
                        "p (two r) -> (p r) two", two=2, r=128))
            for s in range(2):
                r0 = s * CCR
                nc.sync.dma_start(
                    out=Bh[s],
                    in_=ccout[r0:r0 + C, :].rearrange(
                        "(k2 pl p) c -> p k2 pl c", k2=K2, pl=2, p=128))
            nc.vector.tensor_add(Bbf, Bh[0], Bh[1])
            nc.scalar.activation(out=B8, in_=Bbf, func=AF.Identity)
            for s in range(2):
                r0 = s * CCR + C
                nc.scalar.dma_start(
                    out=B2h[s],
                    in_=ccout[r0:r0 + C, :].rearrange(
                        "(k2 pl p) c -> p k2 pl c", k2=K2, pl=2, p=128))
            nc.vector.tensor_add(B2bf, B2h[0], B2h[1])
            nc.scalar.activation(out=B28, in_=B2bf, func=AF.Identity)
            # per-channel rows -> partition-major scalars (gpsimd queue)
            hsh = [const.tile([128, CB], BF16, tag=f"hsh{s}", name=f"hsh{s}")
                   for s in range(2)]
            h2sh = [const.tile([128, CB], BF16, tag=f"h2sh{s}",
                               name=f"h2sh{s}") for s in range(2)]
            bnh = [const.tile([128, 2 * CB], BF16, tag=f"bnh{s}",
                              name=f"bnh{s}") for s in range(2)]
            for s in range(2):
                r0 = s * CCR + 2 * C
                nc.gpsimd.dma_start(
                    out=hsh[s], in_=ccout[r0:r0 + 1, :].rearrange(
                        "one (cb p) -> (one p) cb", cb=CB, p=128))
                nc.gpsimd.dma_start(
                    out=h2sh[s], in_=ccout[r0 + 1:r0 + 2, :].rearrange(
                        "one (cb p) -> (one p) cb", cb=CB, p=128))
                nc.gpsimd.dma_start(
                    out=bnh[s], in_=ccout[r0 + 3:r0 + 5, :].rearrange(
                        "a (p c) -> (a p) c", p=64, c=2 * CB))
            nc.vector.tensor_add(hs_sb, hsh[0], hsh[1])
            nc.vector.tensor_add(h2s_sb, h2sh[0], h2sh[1])
            bn_f = small.tile([128, 2 * CB], F32, tag="bnf", name="bnf")
            nc.vector.tensor_add(bn_f, bnh[0], bnh[1])

            # ---- den chains: their latency overlaps the readback ----
            Mc = float(N)
            ivbcs = []
            for pc in range(PC):
                psl = slice(pc * NC, (pc + 1) * NC)
                dps = pp.tile([1, NC], F32, tag="ps", name="dps")
                for k2 in range(K2):
                    nc.tensor.matmul(dps, u2t[:, :, k2:k2 + 1],
                                     FS[k2][:, :, psl], perf_mode=DR,
                                     start=(k2 == 0), stop=(k2 == K2 - 1))
                den = ivp.tile([1, NC], F32, tag=f"den{pc}", name="den")
                nc.vector.tensor_scalar_add(den, dps, Mc + EPS)
                ivd = ivp.tile([1, NC], F32, tag=f"ivd{pc}", name="ivd")
                nc.vector.reciprocal_approx_fast(out=ivd, in_=den)
                ivbc = ivp.tile([128, NC], F32, tag=f"ivbc{pc}", name="ivbc")
                nc.gpsimd.partition_broadcast(ivbc, ivd)
                ivbcs.append(ivbc)

            # ---- merge bn stats: mean/var over the full image ----
            mm4 = small.tile([128, 16], F32, tag="bmm", name="bmm")
            nc.gpsimd.tensor_mul(mm4[:, 0:CB], bn_f[:, 0:CB], bn_f[:, 0:CB])
            varb = small.tile([128, 16], F32, tag="bvar", name="bvar")
            nc.vector.tensor_sub(varb[:, 0:CB], bn_f[:, CB:2 * CB],
                                 mm4[:, 0:CB])
            nc.vector.tensor_copy(out=cmean[:, 0:CB], in_=bn_f[:, 0:CB])
            cstd = small.tile([128, 16], F32, tag="cstd", name="cstd")
            nc.scalar.activation(out=cstd[:, 0:CB], in_=varb[:, 0:CB],
                                 func=AF.Sqrt, bias=eps_sb[:, 0:1],
                                 scale=float(NT) / (NT - 1))
            nc.vector.reciprocal_approx_fast(out=cinv[:, 0:CB],
                                             in_=cstd[:, 0:CB])
            nc.vector.tensor_mul(negmc[:, 0:CB], cmean[:, 0:CB],
                                 cinv[:, 0:CB])
            nc.vector.tensor_scalar_mul(negmc[:, 0:CB], negmc[:, 0:CB], -1.0)

            # ---- main loop: mean/sq from reduced B, B2 + AdaIN combine.
            #      PSUM evacs are per-bank STTs into halves of [128, 1024]
            #      tiles; the rest of the combine runs 1024-wide to halve
            #      per-op overhead.  (+bh is applied on the host.) ----
            for pw in range(PW):
                wsl = slice(pw * 2 * NC, (pw + 1) * 2 * NC)
                for cb in range(CB):
                    csl = slice(cb * 128, (cb + 1) * 128)
                    if pw == 0:
                        # normalize content in place: CT <- (ct - cmean)/cstd
                        nc.scalar.activation(out=CT[cb], in_=CT[cb],
                                             func=AF.Identity,
                                             scale=cinv[:, cb:cb + 1],
                                             bias=negmc[:, cb:cb + 1])
                    mean_t = cmb.tile([128, 2 * NC], BF16, tag="mean",
                                      name="mean")
                    sqs_t = cmb.tile([128, 2 * NC], BF16, tag="sqs",
                                     name="sqs")
                    for half in range(2):
                        pc = pw * 2 + half
                        psl = slice(pc * NC, (pc + 1) * NC)
                        dsl = slice(half * NC, (half + 1) * NC)
                        psm = pp.tile([128, NC], F32, tag="ps", name="psm")
                        for k2 in range(K2):
                            nc.tensor.matmul(psm, B8[:, k2, :, csl],
                                             FS[k2][:, :, psl], perf_mode=DR,
                                             start=(k2 == 0),
                                             stop=(k2 == K2 - 1))
                        pss = pp.tile([128, NC], F32, tag="ps", name="pss")
                        for k2 in range(K2):
                            nc.tensor.matmul(pss, B28[:, k2, :, csl],
                                             FS[k2][:, :, psl], perf_mode=DR,
                                             start=(k2 == 0),
                                             stop=(k2 == K2 - 1))
                        # mean = (hsum + B^T F') / den ; sq likewise
                        nc.vector.scalar_tensor_tensor(
                            mean_t[:, dsl], psm, hs_sb[:, cb:cb + 1],
                            ivbcs[pc], op0=ALU.add, op1=ALU.mult)
                        nc.vector.scalar_tensor_tensor(
                            sqs_t[:, dsl], pss, h2s_sb[:, cb:cb + 1],
                            ivbcs[pc], op0=ALU.add, op1=ALU.mult)
                    m2_t = cmb.tile([128, 2 * NC], BF16, tag="m2", name="m2")
                    nc.scalar.activation(out=m2_t, in_=mean_t, func=AF.Square)
                    nc.gpsimd.tensor_sub(sqs_t, sqs_t, m2_t)
                    sd_t = cmb.tile([128, 2 * NC], BF16, tag="sd", name="sd")
                    nc.scalar.activation(out=sd_t, in_=sqs_t, func=AF.Sqrt)
                    out_t = op.tile([128, 2 * NC], BF16, tag="out",
                                    name="out_t")
                    nc.vector.tensor_mul(out_t, sd_t, CT[cb][:, wsl])
                    nc.vector.tensor_add(out_t, out_t, mean_t)
                    nc.sync.dma_start(out=out[csl, wsl], in_=out_t)

    nc.finalize()
    return nc


_NC_CACHE = {}


def _get_nc(C, N, NL):
    key = (C, N, NL)
    if key not in _NC_CACHE:
        _NC_CACHE[key] = build_nc(C, N, NL)
    return _NC_CACHE[key]


def make_in_maps(content, style, content_key, style_key, Wf, bf, Wg, bg, Wh, bh):
    """Shard full inputs into 8 per-core input maps."""
    B, C, H, W = content.shape
    NP = H * W
    NL = NP // 2
    KB = C // 128

    def prep(x):
        return np.ascontiguousarray(x, dtype=np.float32)

    def prep16(x):
        return np.ascontiguousarray(np.asarray(x).astype(ml_dtypes.bfloat16))

    def prep8i(x):  # [C, n] -> [128, KB//2, 2, n] fp8 DoubleRow interleave
        Cd, n = x.shape
        k2 = Cd // 256
        return np.ascontiguousarray(
            np.asarray(x).reshape(k2, 2, 128, n).transpose(2, 0, 1, 3)
        ).astype(ml_dtypes.float8_e4m3)

    wfT = prep8i(np.asarray(Wf).T)
    wgT = prep8i(np.asarray(Wg).T)
    whT = prep8i(np.asarray(Wh).T)
    bfb = prep(np.asarray(bf).reshape(KB, 128).T)
    bgrr = prep16(np.asarray(bg).reshape(1, C))

    in_maps = []
    for core in range(8):
        b, h = core // 2, core % 2
        hsl = slice(h * NL, (h + 1) * NL)
        in_maps.append({
            "ck": prep8i(np.asarray(content_key[b]).reshape(C, NP)[:, hsl]),
            "sk": prep8i(np.asarray(style_key[b]).reshape(C, NP)[:, hsl]),
            "st": prep8i(np.asarray(style[b]).reshape(C, NP)[:, hsl]),
            "ct": prep16(np.asarray(content[b]).reshape(C, NP)[:, hsl]),
            "wf": wfT, "wg": wgT, "wh": whT,
            "bfb": bfb, "bgr": bgrr,
        })
    return in_maps


def kernel(content, style, content_key, style_key, Wf, bf, Wg, bg, Wh, bh,
           _trace=False):
    B, C, H, W = content.shape
    NP = H * W
    NL = NP // 2
    nc = _get_nc(C, NP, NL)
    in_maps = make_in_maps(content, style, content_key, style_key,
                           Wf, bf, Wg, bg, Wh, bh)
    res = run_bass_kernel_spmd(nc, in_maps, core_ids=list(range(8)), trace=_trace)
    out = np.empty((B, C, NP), dtype=np.float32)
    for core in range(8):
        b, h = core // 2, core % 2
        out[b, :, h * NL:(h + 1) * NL] = res.results[core]["out"]
    # the conv bias bh shifts mean only (it cancels inside std): add it here
    out += np.asarray(bh, dtype=np.float32)[None, :, None]
    if _trace:
        kernel.last_results = res
    return out.reshape(B, C, H, W)
